# revision 1
# baseline (speedup 1.0000x reference)
"""Trainium2 Bass kernel for nn_ExpendMemoryUnit (scatter_memory).

Sharding: 8 cores = 4 pairs; pair p handles sample b=p. Within a pair the
full-attention j-dimension (keys/values) is split in half; the partial
unnormalized attention output + softmax denominators are summed with one
pairwise AllReduce; the cheap tail (diff/conv64/InstanceNorm) is duplicated
in the pair. All matmuls run in bf16 with fp32 PSUM accumulation.
"""

import math
import numpy as np
import ml_dtypes
from contextlib import ExitStack

import concourse.bacc as bacc
import concourse.tile as tile
from concourse import mybir
from concourse.bass_utils import run_bass_kernel_spmd

FP32 = mybir.dt.float32
BF16 = mybir.dt.bfloat16
AF = mybir.ActivationFunctionType
ALU = mybir.AluOpType
bf = ml_dtypes.bfloat16

C = 128
S = 64
HW = S * S            # 4096
JL = HW // 2          # 2048 local keys per core
B = 4
NCORES = 8
PW = S + 2            # padded row width 66
INV1152 = 1.0 / math.sqrt(C * 9.0)

_CACHE = {}


def _taps():
    for dy in range(3):
        for dx in range(3):
            yield dy * 3 + dx, dy, dx


def _build_program(n_iters=1, skip_collective=False):
    nc = bacc.Bacc("TRN2", target_bir_lowering=False, debug=False,
                   num_devices=NCORES)

    def inp(name, shape, dtype):
        return nc.dram_tensor(name, list(shape), dtype, kind="ExternalInput").ap()

    # per-core inputs
    x_cs = inp("x_cs", [C, HW], BF16)
    x_kv = inp("x_kv", [C, 34 * S], BF16)
    kp = inp("kp", [30, 1], BF16)
    # replicated inputs
    mb16 = inp("mb16", [C, HW], BF16)
    scw9T = inp("scw9T", [C, 9 * C], BF16)
    s2T = inp("s2T", [C, C], FP32)
    projT = inp("projT", [C, C], BF16)
    qwT = inp("qwT", [C, C], BF16)
    diffT = inp("diffT", [C, C], BF16)
    c64T = inp("c64T", [C, 9 * C], BF16)
    dynT = inp("dynT", [C, 4 * 9 * 256], BF16)
    dynb2 = inp("dynb2", [C, 8], FP32)
    rw1aT = inp("rw1aT", [C, C], BF16)
    rw1bT = inp("rw1bT", [30, C], BF16)
    rw2T = inp("rw2T", [C, 512], BF16)
    modT = inp("modT", [C, 512], BF16)
    att1T = inp("att1T", [C, 33], BF16)
    att2T = inp("att2T", [33, 4], BF16)
    ident = inp("ident", [C, C], BF16)
    pb = inp("pb", [C, 1], FP32)
    rb1 = inp("rb1", [C, 1], FP32)
    rb2 = inp("rb2", [C, 4], FP32)
    modb = inp("modb", [C, 1], FP32)
    actb = inp("actb", [C, 1], FP32)
    qb = inp("qb", [C, 1], FP32)
    diffb = inp("diffb", [C, 1], FP32)
    ing = inp("ing", [C, 1], FP32)
    inb = inp("inb", [C, 1], FP32)
    attb2 = inp("attb2", [1, 4], FP32)

    y = nc.dram_tensor("y", [C, HW], FP32, kind="ExternalOutput").ap()

    with tile.TileContext(nc) as tc, ExitStack() as ctx:
        sb = ctx.enter_context(tc.tile_pool(name="sb", bufs=1))
        st = ctx.enter_context(tc.tile_pool(name="st", bufs=2))
        ps = ctx.enter_context(tc.tile_pool(name="ps", bufs=1, space="PSUM"))
        dram = ctx.enter_context(tc.tile_pool(name="dram", bufs=1, space="DRAM"))

        def sbt(name, shape, dtype, **kw):
            return sb.tile(shape, dtype, name=name, **kw)

        # ---- load everything into SBUF (x_cs chunked so feat conv starts early)
        projT_sb0 = sb.tile_from(projT)
        pb_sb0 = sb.tile_from(pb)
        xcs_sb = sb.tile([C, HW], BF16, name="xcs_sb")
        for _ib in range(4):
            nc.sync.dma_start(xcs_sb[:, _ib * 1024:(_ib + 1) * 1024],
                              x_cs[:, _ib * 1024:(_ib + 1) * 1024])
        projT_sb = projT_sb0
        pb_sb = pb_sb0
        # small early-need weights before the big late-need tensors
        qwT_sb = sb.tile_from(qwT)
        qb_sb = sb.tile_from(qb)
        kp_sb = sb.tile_from(kp)
        rw1aT_sb = sb.tile_from(rw1aT)
        rw1bT_sb = sb.tile_from(rw1bT)
        rw2T_sb = sb.tile_from(rw2T)
        modT_sb = sb.tile_from(modT)
        s2T_sb = sb.tile_from(s2T)
        rb1_sb = sb.tile_from(rb1)
        rb2_sb = sb.tile_from(rb2)
        modb_sb = sb.tile_from(modb)
        actb_sb = sb.tile_from(actb)
        xkv_sb = sb.tile_from(x_kv)
        scw9T_sb = sb.tile_from(scw9T)
        mb_sb = sb.tile_from(mb16)
        att1T_sb = sb.tile_from(att1T)
        att2T_sb = sb.tile_from(att2T)
        attb2_sb = sb.tile_from(attb2)
        dynb2_sb = sb.tile_from(dynb2)
        dynT_sb = sb.tile_from(dynT)
        ident_sb = sb.tile_from(ident)
        diffT_sb = sb.tile_from(diffT)
        diffb_sb = sb.tile_from(diffb)
        c64T_sb = sb.tile_from(c64T)
        ing_sb = sb.tile_from(ing)
        inb_sb = sb.tile_from(inb)

        ones_bf = sbt("ones_bf", [C, 1], BF16)
        nc.vector.memset(ones_bf[:], 1.0)
        ones_row = sbt("ones_row", [1, C], FP32)
        nc.vector.memset(ones_row[:], 1.0)
        eps8 = sbt("eps8", [C, 1], FP32)
        nc.vector.memset(eps8[:], 1e-8)
        eps5 = sbt("eps5", [C, 1], FP32)
        nc.vector.memset(eps5[:], 1e-5)

        def _emit_iter():
            # pre-zero padded scratch + copy the memory bank while engines idle
            fkv_pad = sbt("fkv_pad", [C, 34 * PW], BF16)
            nc.gpsimd.memset(fkv_pad[:], 0.0)
            mb_pad = sbt("mb_pad", [C, (S + 2) * PW], BF16)
            nc.gpsimd.memset(mb_pad[:], 0.0)
            mp3 = mb_pad[:].rearrange("p (r c) -> p r c", c=PW)
            nc.vector.tensor_copy(mp3[:, 1:1 + S, 1:1 + S],
                                  mb_sb[:].rearrange("p (r c) -> p r c", c=S))
            c64in_pad = sbt("c64in_pad", [C, (S + 2) * PW], BF16)
            nc.gpsimd.memset(c64in_pad[:], 0.0)

            # ---- phase A: feat conv (1x1) from x_cs; per-channel sums for style
            feat_bf = sbt("feat_bf", [C, HW], BF16)
            fsum4 = sbt("fsum4", [C, 4], FP32)
            for ib in range(4):
                fp = ps.tile([C, 1024], FP32, name="fp", tag="A", bufs=2)
                for s2_ in range(2):
                    sl = slice(ib * 1024 + s2_ * 512, ib * 1024 + s2_ * 512 + 512)
                    nc.tensor.matmul(fp[:, s2_ * 512:s2_ * 512 + 512], projT_sb[:],
                                     xcs_sb[:, sl], start=True, stop=True)
                nc.vector.tensor_scalar(
                    feat_bf[:, ib * 1024:(ib + 1) * 1024], fp[:], pb_sb[:, 0:1],
                    0.0, ALU.add, ALU.add, accum_out=fsum4[:, ib:ib + 1])

            # feat on the kv halo window (34 rows), into a column-padded buffer
            fkv3 = fkv_pad[:].rearrange("p (r c) -> p r c", c=PW)
            for blk, (c0, c1) in enumerate([(0, 1024), (1024, 2048), (2048, 2176)]):
                n = c1 - c0
                kvp = ps.tile([C, 1024], FP32, name="kvp", tag="A", bufs=2)
                for s0 in range(0, n, 512):
                    w = min(512, n - s0)
                    nc.tensor.matmul(kvp[:, s0:s0 + w], projT_sb[:],
                                     xkv_sb[:, c0 + s0:c0 + s0 + w],
                                     start=True, stop=True)
                r0 = c0 // S
                nr = n // S
                nc.vector.tensor_scalar(
                    fkv3[:, r0:r0 + nr, 1:1 + S], kvp[:, 0:n].rearrange(
                        "p (r c) -> p r c", c=S),
                    pb_sb[:, 0:1], None, ALU.add)

            # ---- q conv (independent of style; keeps PE busy)
            qT_sb = sbt("qT_sb", [C, HW], BF16)
            for ib in range(4):
                qp = ps.tile([C, 1024], FP32, name="qp", tag="A", bufs=2)
                for s2_ in range(2):
                    sl = slice(ib * 1024 + s2_ * 512, ib * 1024 + s2_ * 512 + 512)
                    nc.tensor.matmul(qp[:, s2_ * 512:s2_ * 512 + 512], qwT_sb[:],
                                     feat_bf[:, sl], start=True, stop=True)
                nc.vector.tensor_scalar(
                    qT_sb[:, ib * 1024:(ib + 1) * 1024], qp[:], qb_sb[:, 0:1],
                    0.0, ALU.add, ALU.max)

            # ---- style MLP chain (tiny)
            fsum1 = sbt("fsum1", [C, 1], FP32)
            nc.vector.reduce_sum(fsum1[:], fsum4[:], axis=mybir.AxisListType.X)
            fsum_bf = sbt("fsum_bf", [C, 1], BF16)
            nc.vector.tensor_copy(fsum_bf[:], fsum1[:])
            h1ps = ps.tile([C, 1], FP32, name="h1ps", tag="CC")
            nc.tensor.matmul(h1ps[:], rw1aT_sb[:], fsum_bf[:], start=True, stop=False)
            nc.tensor.matmul(h1ps[:], rw1bT_sb[:], kp_sb[:], start=False, stop=True)
            h1_bf = sbt("h1_bf", [C, 1], BF16)
            nc.vector.tensor_scalar(h1_bf[:], h1ps[:], rb1_sb[:, 0:1], 0.0,
                                    ALU.add, ALU.max)
            scps = ps.tile([C, 4], FP32, name="scps", tag="CC")
            for c4 in range(4):
                nc.tensor.matmul(scps[:, c4:c4 + 1], rw2T_sb[:, c4 * C:(c4 + 1) * C],
                                 h1_bf[:], start=True, stop=True)
            sc_bf = sbt("sc_bf", [C, 4], BF16)
            nc.vector.tensor_tensor(sc_bf[:], scps[:], rb2_sb[:], ALU.add)
            styps = ps.tile([C, 1], FP32, name="styps", tag="CC")
            for c4 in range(4):
                nc.tensor.matmul(styps[:], modT_sb[:, c4 * C:(c4 + 1) * C],
                                 sc_bf[:, c4:c4 + 1], start=(c4 == 0), stop=(c4 == 3))
            style_f = sbt("style_f", [C, 1], FP32)
            nc.vector.tensor_scalar(style_f[:], styps[:], modb_sb[:, 0:1], None, ALU.add)
            # demod = rsqrt(sum_i style_i^2 * S2[i, o] + 1e-8)
            st2 = sbt("st2", [C, 1], FP32)
            nc.vector.tensor_tensor(st2[:], style_f[:], style_f[:], ALU.mult)
            s2ps = ps.tile([C, 1], FP32, name="s2ps", tag="CC")
            nc.tensor.matmul(s2ps[:], s2T_sb[:], st2[:], start=True, stop=True)
            # demod = rsqrt(x + 1e-8) = exp(-0.5 * ln(x + 1e-8)); keeps ACT on
            # the natural_log_exp table set the attention exps already need
            sdp = sbt("sdp", [C, 1], FP32)
            nc.scalar.activation(sdp[:], s2ps[:], AF.Ln, bias=eps8[:, 0:1])
            demod = sbt("demod", [C, 1], FP32)
            nc.scalar.activation(demod[:], sdp[:], AF.Exp, scale=-0.5)

            # ---- modulated 3x3 conv on the memory bank (mean of leaky only)
            # style modulation folded into the 9 tap weights, so mb_pad is a
            # plain copy that loads straight from DRAM at kernel start
            scw9s = sbt("scw9s", [C, 9 * C], BF16)
            nc.vector.tensor_scalar(scw9s[:], scw9T_sb[:], style_f[:, 0:1],
                                    None, ALU.mult)
            lksum4 = sbt("lksum4", [C, 4], FP32)
            for ib in range(4):
                mps = ps.tile([C, 1024], FP32, name="mps", tag="A", bufs=2)
                for s2_ in range(2):
                    for d, dy, dx in _taps():
                        r0 = ib * 16 + s2_ * 8
                        rhs = mp3[:, r0 + dy:r0 + dy + 8, dx:dx + S]
                        nc.tensor.matmul(mps[:, s2_ * 512:s2_ * 512 + 512],
                                         scw9s[:, d * C:(d + 1) * C], rhs,
                                         start=(d == 0), stop=(d == 8))
                ytmp = st.tile([C, 1024], FP32, name="ytmp")
                nc.vector.tensor_scalar(ytmp[:], mps[:], demod[:, 0:1],
                                        actb_sb[:, 0:1], ALU.mult, ALU.add)
                lk = st.tile([C, 1024], FP32, name="lk")
                nc.vector.scalar_tensor_tensor(
                    lk[:], ytmp[:], 0.2, ytmp[:], ALU.mult, ALU.max,
                    accum_out=lksum4[:, ib:ib + 1])

            # ---- attention2d routing -> att weights, broadcast to all partitions
            lksum1 = sbt("lksum1", [C, 1], FP32)
            nc.vector.reduce_sum(lksum1[:], lksum4[:], axis=mybir.AxisListType.X)
            a_bf = sbt("a_bf", [C, 1], BF16)
            nc.vector.tensor_copy(a_bf[:], lksum1[:])
            ahps = ps.tile([33, 1], FP32, name="ahps", tag="CC")
            nc.tensor.matmul(ahps[:], att1T_sb[:], a_bf[:], start=True, stop=True)
            ah_bf = sbt("ah_bf", [33, 1], BF16)
            nc.vector.tensor_scalar(ah_bf[:], ahps[:], 0.0, 0.0, ALU.add, ALU.max)
            attps = ps.tile([1, 4], FP32, name="attps", tag="CC")
            nc.tensor.matmul(attps[:], ah_bf[:], att2T_sb[:], start=True, stop=True)
            attl = sbt("attl", [1, 4], FP32)
            nc.vector.tensor_tensor(attl[:], attps[:], attb2_sb[:], ALU.add)
            atte = sbt("atte", [1, 4], FP32)
            attsum = sbt("attsum", [1, 1], FP32)
            nc.scalar.activation(atte[:], attl[:], AF.Exp, scale=1.0 / 34.0,
                                 accum_out=attsum[:])
            attr = sbt("attr", [1, 1], FP32)
            nc.vector.reciprocal(attr[:], attsum[:])
            att_row = sbt("att_row", [1, 4], FP32)
            nc.vector.tensor_scalar(att_row[:], atte[:], attr[:, 0:1], None, ALU.mult)
            abps = ps.tile([C, 4], FP32, name="abps", tag="CC")
            nc.tensor.matmul(abps[:], ones_row[:], att_row[:], start=True, stop=True)
            att_bc = sbt("att_bc", [C, 4], FP32)
            nc.vector.tensor_copy(att_bc[:], abps[:])

            # ---- aggregate expert conv weights (DVE), in 3 tap-chunks of 768
            agg_wT = sbt("agg_wT", [C, 9 * 256], BF16)
            for g in range(3):
                sl0 = g * 768
                acc = st.tile([C, 768], FP32, name="aggacc")
                nc.vector.tensor_scalar(acc[:], dynT_sb[:, sl0:sl0 + 768],
                                        att_bc[:, 0:1], None, ALU.mult)
                for k in range(1, 4):
                    acc2 = st.tile([C, 768], FP32, name="aggacc")
                    nc.vector.scalar_tensor_tensor(
                        acc2[:], dynT_sb[:, k * 2304 + sl0:k * 2304 + sl0 + 768],
                        att_bc[:, k:k + 1], acc[:], ALU.mult, ALU.add)
                    acc = acc2
                nc.vector.tensor_copy(agg_wT[:, sl0:sl0 + 768], acc[:])
            ab = sbt("ab", [C, 2], FP32)
            abx = st.tile([C, 2], FP32, name="abx")
            nc.vector.tensor_scalar(abx[:], dynb2_sb[:, 0:2], att_bc[:, 0:1],
                                    None, ALU.mult)
            for k in range(1, 4):
                abx2 = st.tile([C, 2], FP32, name="abx")
                nc.vector.scalar_tensor_tensor(
                    abx2[:], dynb2_sb[:, 2 * k:2 * k + 2], att_bc[:, k:k + 1],
                    abx[:], ALU.mult, ALU.add)
                abx = abx2
            nc.vector.tensor_copy(ab[:], abx[:])

            # ---- kv conv (3x3, per-sample weights) on the local 32-row window
            kT_sb = sbt("kT_sb", [C, JL], BF16)
            vT_sb = sbt("vT_sb", [C, JL], BF16)
            v_sb = sbt("v_sb", [C, JL], BF16)
            for ib in range(2):
                for half, dst, bcol in ((0, kT_sb, 0), (1, vT_sb, 1)):
                    cps = ps.tile([C, 1024], FP32, name="cps", tag="A", bufs=2)
                    for s2_ in range(2):
                        for d, dy, dx in _taps():
                            lhsT = agg_wT[:, d * 256 + half * C:d * 256 + (half + 1) * C]
                            r0 = ib * 16 + s2_ * 8
                            rhs = fkv3[:, r0 + dy:r0 + dy + 8, dx:dx + S]
                            nc.tensor.matmul(cps[:, s2_ * 512:s2_ * 512 + 512],
                                             lhsT, rhs, start=(d == 0), stop=(d == 8))
                    nc.vector.tensor_scalar(
                        dst[:, ib * 1024:(ib + 1) * 1024], cps[:],
                        ab[:, bcol:bcol + 1], 0.0, ALU.add, ALU.max)
                # transpose this block's v: v_sb[:, t*128:(t+1)*128] = vT.T
                for g in range(2):
                    tp = ps.tile([C, 4 * C], BF16, name="tp", tag="B")
                    t0 = ib * 8 + g * 4
                    for k in range(4):
                        nc.tensor.transpose(
                            tp[:, k * C:(k + 1) * C],
                            vT_sb[:, (t0 + k) * C:(t0 + k + 1) * C], ident_sb[:])
                    nc.vector.tensor_copy(v_sb[:, t0 * C:(t0 + 4) * C], tp[:])

            # ---- flash-style attention over local keys (j), full queries (i)
            oouts = []
            for ib in range(4):
                ops_t = ps.tile([C, 1024], FP32, name="ops_t", tag="B")
                dps_t = ps.tile([1, 1024], FP32, name="dps_t", tag="CC")
                for jt in range(16):
                    stp = ps.tile([C, 1024], FP32, name="stp", tag="A", bufs=2)
                    for s2_ in range(2):
                        sl = slice(ib * 1024 + s2_ * 512, ib * 1024 + s2_ * 512 + 512)
                        nc.tensor.matmul(stp[:, s2_ * 512:s2_ * 512 + 512],
                                         kT_sb[:, jt * C:(jt + 1) * C],
                                         qT_sb[:, sl], start=True, stop=True)
                    pt = st.tile([C, 1024], BF16, name="pt", bufs=6)
                    nc.scalar.activation(pt[:], stp[:], AF.Exp, scale=0.25)
                    for s2_ in range(2):
                        sl2 = slice(s2_ * 512, s2_ * 512 + 512)
                        nc.tensor.matmul(ops_t[:, sl2], v_sb[:, jt * C:(jt + 1) * C],
                                         pt[:, sl2], start=(jt == 0), stop=(jt == 15))
                    for s2_ in range(2):
                        sl2 = slice(s2_ * 512, s2_ * 512 + 512)
                        nc.tensor.matmul(dps_t[:, sl2], ones_bf[:], pt[:, sl2],
                                         start=(jt == 0), stop=(jt == 15))
                o_stage = st.tile([C, 1024], BF16, name="o_stage", bufs=3)
                nc.vector.tensor_copy(o_stage[:], ops_t[:])
                d_stage = st.tile([1, 1024], BF16, name="d_stage")
                nc.vector.tensor_copy(d_stage[:], dps_t[:])
                oc = dram.tile([C + 1, 1024], BF16, name=f"oacc{ib}")
                nc.sync.dma_start(oc[0:C, :], o_stage[:])
                nc.sync.dma_start(oc[C:C + 1, :], d_stage[:])
                oout = dram.tile([C + 1, 1024], BF16, name=f"oaccout{ib}")
                if skip_collective:
                    nc.sync.dma_start(oout[:], oc[:])
                else:
                    nc.gpsimd.collective_compute(
                        "AllReduce", ALU.add,
                        replica_groups=[[0, 1], [2, 3], [4, 5], [6, 7]],
                        ins=[oc[:]], outs=[oout[:]],
                    )
                oouts.append(oout)

            o_f32 = sbt("o_f32", [C, HW], BF16)
            den32 = sbt("den32", [C, 32], BF16)
            rec32 = sbt("rec32", [C, 32], FP32)
            rec32b = sbt("rec32b", [C, 32], BF16)
            recd = dram.tile([HW], BF16, name="recd")
            rdbc = sbt("rdbc", [C, HW], BF16)
            for q in range(4):
                oout = oouts[q]
                qp_ = slice(q * 32, (q + 1) * 32)
                nc.sync.dma_start(o_f32[:, q * 1024:(q + 1) * 1024],
                                  oout[0:C, :])
                nc.sync.dma_start(
                    den32[qp_, :],
                    oout[C, :].rearrange("(p c) -> p c", c=32))
                nc.vector.reciprocal(rec32[qp_, :], den32[qp_, :])
                nc.vector.tensor_copy(rec32b[qp_, :], rec32[qp_, :])
                nc.sync.dma_start(
                    recd[q * 1024:(q + 1) * 1024].rearrange(
                        "(p c) -> p c", c=32), rec32b[qp_, :])
            for ib in range(4):
                sl = slice(ib * 1024, (ib + 1) * 1024)
                nc.sync.dma_start(rdbc[:, sl], recd[sl].partition_broadcast(C))

            # ---- residual normalize, diff conv, conv64 input
            dconv_in = sbt("dconv_in", [C, HW], BF16)
            for ib in range(4):
                sl = slice(ib * 1024, (ib + 1) * 1024)
                rn = st.tile([C, 1024], BF16, name="rn", bufs=3)
                nc.vector.scalar_tensor_tensor(rn[:], o_f32[:, sl], -1.0,
                                               rdbc[:, sl], ALU.mult, ALU.mult)
                nc.vector.tensor_tensor(dconv_in[:, sl], rn[:], feat_bf[:, sl],
                                        ALU.add)
            cp3 = c64in_pad[:].rearrange("p (r c) -> p r c", c=PW)
            feat3 = feat_bf[:].rearrange("p (r c) -> p r c", c=S)
            for ib in range(4):
                dps2 = ps.tile([C, 1024], FP32, name="dps2", tag="A", bufs=2)
                for s2_ in range(2):
                    sl = slice(ib * 1024 + s2_ * 512, ib * 1024 + s2_ * 512 + 512)
                    nc.tensor.matmul(dps2[:, s2_ * 512:s2_ * 512 + 512], diffT_sb[:],
                                     dconv_in[:, sl], start=True, stop=True)
                dtmp = st.tile([C, 1024], BF16, name="dtmp", bufs=3)
                nc.vector.tensor_scalar(dtmp[:], dps2[:], diffb_sb[:, 0:1], 0.0,
                                        ALU.add, ALU.max)
                nc.vector.tensor_tensor(
                    cp3[:, 1 + ib * 16:1 + (ib + 1) * 16, 1:1 + S],
                    dtmp[:].rearrange("p (r c) -> p r c", c=S),
                    feat3[:, ib * 16:(ib + 1) * 16, :], ALU.add)

            # ---- conv64 (3x3) + InstanceNorm + relu
            stats48 = sbt("stats48", [C, 48], FP32)
            cpsums = []
            for ch, tag in ((0, "A"), (1, "A"), (2, "B"), (3, "CC")):
                bufs = 2 if tag == "A" else None
                cp = ps.tile([C, 1024], FP32, name=f"c64p{ch}", tag=tag, bufs=bufs)
                for s2_ in range(2):
                    for d, dy, dx in _taps():
                        r0 = ch * 16 + s2_ * 8
                        rhs = cp3[:, r0 + dy:r0 + dy + 8, dx:dx + S]
                        nc.tensor.matmul(cp[:, s2_ * 512:s2_ * 512 + 512],
                                         c64T_sb[:, d * C:(d + 1) * C], rhs,
                                         start=(d == 0), stop=(d == 8))
                for s2_ in range(2):
                    nc.vector.bn_stats(
                        stats48[:, (ch * 2 + s2_) * 6:(ch * 2 + s2_ + 1) * 6],
                        cp[:, s2_ * 512:s2_ * 512 + 512])
                cpsums.append(cp)
            mv = sbt("mv", [C, 2], FP32)
            nc.vector.bn_aggr(mv[:], stats48[:].rearrange("p (g k) -> p g k", k=6))
            sd2 = sbt("sd2", [C, 1], FP32)
            nc.scalar.activation(sd2[:], mv[:, 1:2], AF.Ln, bias=eps5[:, 0:1])
            rsig = sbt("rsig", [C, 1], FP32)
            nc.scalar.activation(rsig[:], sd2[:], AF.Exp, scale=-0.5)
            nsc = sbt("nsc", [C, 1], FP32)
            nc.vector.tensor_tensor(nsc[:], rsig[:], ing_sb[:], ALU.mult)
            nt = sbt("nt", [C, 1], FP32)
            nc.vector.tensor_tensor(nt[:], mv[:, 0:1], nsc[:], ALU.mult)
            nbias = sbt("nbias", [C, 1], FP32)
            nc.vector.tensor_tensor(nbias[:], inb_sb[:], nt[:], ALU.subtract)
            for ch in range(4):
                ysb = st.tile([C, 1024], FP32, name="ysb", bufs=3)
                nc.scalar.activation(ysb[:], cpsums[ch][:], AF.Relu,
                                     bias=nbias[:, 0:1], scale=nsc[:, 0:1])
                nc.sync.dma_start(y[:, ch * 1024:(ch + 1) * 1024], ysb[:])


        for _it in range(n_iters):
            _emit_iter()

    nc.compile()
    return nc


def _host_prepare(inputs):
    f32 = np.float32
    feature = np.asarray(inputs["feature"], f32)
    keypoints = np.asarray(inputs["keypoints"], f32)
    mb = np.asarray(inputs["mb"], f32)

    scw = np.asarray(inputs["sc_weight"], f32)[0] * INV1152   # [C,C,3,3] o,i,dy,dx
    rep = {}
    rep["mb16"] = mb[0].reshape(C, HW).astype(bf)
    rep["scw9T"] = np.ascontiguousarray(
        scw.transpose(1, 2, 3, 0).reshape(C, 9 * C)).astype(bf)  # [i,(dy dx) o]
    rep["s2T"] = np.ascontiguousarray(
        (scw ** 2).sum(axis=(2, 3)).T).astype(f32)               # [i, o]
    rep["projT"] = np.ascontiguousarray(
        np.asarray(inputs["proj_w"], f32)[:, :, 0, 0].T).astype(bf)
    rep["qwT"] = np.ascontiguousarray(
        np.asarray(inputs["q_w"], f32)[:, :, 0, 0].T).astype(bf)
    rep["diffT"] = np.ascontiguousarray(
        np.asarray(inputs["diff_w"], f32)[:, :, 0, 0].T).astype(bf)
    c64w = np.asarray(inputs["conv64_w"], f32)                   # [o,i,3,3]
    rep["c64T"] = np.ascontiguousarray(
        c64w.transpose(1, 2, 3, 0).reshape(C, 9 * C)).astype(bf)
    dynw = np.asarray(inputs["dyn_w"], f32)                      # [4,256,128,3,3]
    rep["dynT"] = np.ascontiguousarray(
        dynw.transpose(2, 0, 3, 4, 1).reshape(C, 4 * 9 * 256)).astype(bf)
    dynb = np.asarray(inputs["dyn_b"], f32)                      # [4, 256]
    d2 = np.zeros((C, 8), f32)
    for k in range(4):
        for blk in range(2):
            d2[:, 2 * k + blk] = dynb[k, blk * C:(blk + 1) * C]
    rep["dynb2"] = d2
    rw1 = np.asarray(inputs["route_w1"], f32)                    # [128, 158]
    rep["rw1aT"] = np.ascontiguousarray(rw1[:, :C].T / HW).astype(bf)
    rep["rw1bT"] = np.ascontiguousarray(rw1[:, C:].T).astype(bf)
    rw2 = np.asarray(inputs["route_w2"], f32)                    # [512, 128]
    rep["rw2T"] = np.ascontiguousarray(rw2.T).astype(bf)         # [i, m512]
    modw = np.asarray(inputs["mod_w"], f32) / math.sqrt(512.0)   # [128, 512]
    rep["modT"] = np.ascontiguousarray(
        modw.T.reshape(4, C, C).transpose(1, 0, 2).reshape(C, 4 * C)).astype(bf)
    att1 = np.asarray(inputs["att_w1"], f32)[:, :, 0, 0]         # [33, 128]
    rep["att1T"] = np.ascontiguousarray(
        att1.T * (math.sqrt(2.0) / HW)).astype(bf)
    att2 = np.asarray(inputs["att_w2"], f32)[:, :, 0, 0]         # [4, 33]
    rep["att2T"] = np.ascontiguousarray(att2.T).astype(bf)
    rep["ident"] = np.eye(C, dtype=f32).astype(bf)
    col = lambda v: np.asarray(v, f32).reshape(C, 1)
    rep["pb"] = col(inputs["proj_b"])
    rep["rb1"] = col(inputs["route_b1"])
    rb2v = np.asarray(inputs["route_b2"], f32)
    rep["rb2"] = np.ascontiguousarray(rb2v.reshape(4, C).T)      # [m_in, c4]
    rep["modb"] = col(inputs["mod_b"])
    rep["actb"] = col(inputs["act_b"])
    rep["qb"] = col(inputs["q_b"])
    rep["diffb"] = col(inputs["diff_b"])
    rep["ing"] = col(inputs["in_g"])
    rep["inb"] = col(inputs["in_b"])
    rep["attb2"] = np.asarray(inputs["att_b2"], f32).reshape(1, 4)

    per_core = []
    for c in range(NCORES):
        b, h = c // 2, c % 2
        img = feature[b, 1::2]                                   # [C, 64, 64]
        d = {"x_cs": img.reshape(C, HW).astype(bf),
             "kp": keypoints[b].reshape(30, 1).astype(bf)}
        buf = np.zeros((C, 34, S), f32)
        lo = 32 * h - 1
        for j in range(34):
            r = lo + j
            if 0 <= r < S:
                buf[:, j] = img[:, r]
        d["x_kv"] = buf.reshape(C, 34 * S).astype(bf)
        per_core.append(d)
    return rep, per_core


def kernel(**inputs):
    if "nc" not in _CACHE:
        _CACHE["nc"] = _build_program()
    nc = _CACHE["nc"]
    rep, per_core = _host_prepare(inputs)
    in_maps = [{**rep, **pc} for pc in per_core]
    r = run_bass_kernel_spmd(nc, in_maps, core_ids=list(range(NCORES)))
    feature = np.asarray(inputs["feature"], np.float32)
    out = np.empty_like(feature)
    out[:, ::2] = feature[:, ::2]
    for b in range(B):
        out[b, 1::2] = r.results[2 * b]["y"].reshape(C, S, S)
    return out

